# revision 7
# baseline (speedup 1.0000x reference)
"""AdaptiveHeatmapLossFromCenters — Trainium2 Bass kernel (8 NeuronCores).

Math
----
Per sample b (one per core):
  scale_loss = mean(sm^2)
  sizes_n    = (0.2/gr)*(1 + relu(sm[cy_n, cx_n]))        (centers clamped)
  gt[h,w]    = max_n exp(-((h-cy_n)^2+(w-cx_n)^2) / (2 sizes_n^2))
  hm_loss    = mean((hm - gt)^2 * mask)

The max-splat is computed as a power-mean: with g_n the n-th gaussian,
  max_n g_n ≈ (sum_n g_n^K)^(1/K),  K = 12
and g_n^K factorizes per axis, so the whole splat is a matmul:
  p[h,w] = sum_n U[n,h]*V[n,w],  U = exp(K*a_n*(h-cy_n)^2), a_n = -1/(2 s_n^2)
A second moment p2 (2K) gives an Aitken correction (p2/p)^(1/K) that is
exact for m-way ties.  U,V carry a +19.5 exponent shift so p = g^K*e^39 uses
the full f32-normal x Ln-LUT range [1e-38, 2^64] (removed in the final exp).

Outputs per core: gt [512,512] f32 and 8 partial sums (4 tile-sums of sm^2,
4 tile-sums of (hm-gt)^2*mask), scaled by 1/(512*512) on device.  The host
finishes the batch means (the "all-reduce" of the sharding hint).
"""

import math
import os
import sys

import numpy as np

for _p in ("/opt/trn_rl_repo", "/root/.axon_site/_ro/trn_rl_repo"):
    if os.path.isdir(_p) and _p not in sys.path:
        sys.path.insert(0, _p)

import concourse.bacc as bacc
import concourse.bass as bass
import concourse.tile as tile
from concourse import mybir
from concourse.bass_utils import run_bass_kernel_spmd

B = 8
H = W = 512
N = 128
P = 128
NT = H // P  # 4 h-tiles
K = 12.0
SHIFT = 19.5                              # per-factor exponent shift (e^19.5)
UNSHIFT = 39.0                            # combined shift to remove: 2*SHIFT
LN_FLOOR = 1e-37                          # bias inside Ln so ln(0) stays finite
AIT_TH = 1e-8                             # use Aitken branch where p32 > this
INV_HW = 1.0 / float(H * W)

F32 = mybir.dt.float32
BF16 = mybir.dt.bfloat16
I32 = mybir.dt.int32
Alu = mybir.AluOpType
Act = mybir.ActivationFunctionType


def build_nc(ablate=()):
    nc = bacc.Bacc(None, target_bir_lowering=False, debug=False)

    hm_e = nc.dram_tensor("hm", [H, W], F32, kind="ExternalInput")
    sm_e = nc.dram_tensor("sm", [H, W], F32, kind="ExternalInput")
    mk_e = nc.dram_tensor("mask", [H, W], F32, kind="ExternalInput")
    cen_e = nc.dram_tensor("centers", [N, 2], I32, kind="ExternalInput")
    gr_e = nc.dram_tensor("grb", [P, 1], F32, kind="ExternalInput")
    gt_e = nc.dram_tensor("gt", [H, W], F32, kind="ExternalOutput")
    pr_e = nc.dram_tensor("partials", [8, 1], F32, kind="ExternalOutput")

    with tile.TileContext(nc) as tc:
        with (
            tc.tile_pool(name="persist", bufs=1) as pp,
            tc.tile_pool(name="loop", bufs=2) as lp,
            tc.tile_pool(name="psum16", bufs=2, space="PSUM") as ps16,
            tc.tile_pool(name="psum32", bufs=2, space="PSUM") as ps32,
            tc.tile_pool(name="psumfin", bufs=1, space="PSUM") as psf,
        ):
            # ---- bulk input DMAs (per h-tile slices of one big SBUF tile) ----
            smt = pp.tile([P, NT * W], F32, tag="smt")
            hmt = pp.tile([P, NT * W], F32, tag="hmt")
            mkt = pp.tile([P, NT * W], F32, tag="mkt")
            for t in range(NT):
                fs = slice(t * W, (t + 1) * W)
                rs = slice(t * P, (t + 1) * P)
                nc.sync.dma_start(out=smt[:, fs], in_=sm_e[rs, :])
                nc.sync.dma_start(out=hmt[:, fs], in_=hm_e[rs, :])
                nc.sync.dma_start(out=mkt[:, fs], in_=mk_e[rs, :])

            cen = pp.tile([N, 2], I32, tag="cen")
            nc.sync.dma_start(out=cen[:], in_=cen_e[:])
            grb = pp.tile([P, 1], F32, tag="grb")
            nc.sync.dma_start(out=grb[:], in_=gr_e[:])

            # ---- per-center sigma path ----
            cl = pp.tile([N, 2], I32, tag="cl")
            nc.vector.tensor_scalar(
                out=cl[:], in0=cen[:], scalar1=0, scalar2=H - 1,
                op0=Alu.max, op1=Alu.min,
            )
            idx = pp.tile([N, 1], I32, tag="idx")
            nc.vector.scalar_tensor_tensor(
                out=idx[:], in0=cl[:, 0:1], scalar=W, in1=cl[:, 1:2],
                op0=Alu.mult, op1=Alu.add,
            )
            v = pp.tile([N, 1], F32, tag="v")
            if "nogather" in ablate:
                nc.vector.memset(v[:], 0.0)
            else:
                sm_flat = bass.AP(sm_e, 0, [[1, H * W], [1, 1]])
                nc.gpsimd.indirect_dma_start(
                    out=v[:], out_offset=None, in_=sm_flat,
                    in_offset=bass.IndirectOffsetOnAxis(ap=idx[:, 0:1], axis=0),
                )

            rec = pp.tile([P, 1], F32, tag="rec")
            nc.vector.reciprocal(rec[:], grb[:])
            rs_ = pp.tile([P, 1], F32, tag="rs_")
            nc.vector.tensor_scalar(out=rs_[:], in0=rec[:], scalar1=0.2,
                                    scalar2=None, op0=Alu.mult)
            vr = pp.tile([P, 1], F32, tag="vr")
            nc.vector.tensor_scalar(out=vr[:], in0=v[:], scalar1=0.0,
                                    scalar2=1.0, op0=Alu.max, op1=Alu.add)
            sg = pp.tile([P, 1], F32, tag="sg")
            nc.vector.tensor_tensor(out=sg[:], in0=vr[:], in1=rs_[:], op=Alu.mult)
            sg2 = pp.tile([P, 1], F32, tag="sg2")
            nc.vector.tensor_tensor(out=sg2[:], in0=sg[:], in1=sg[:], op=Alu.mult)
            is2 = pp.tile([P, 1], F32, tag="is2")
            nc.vector.reciprocal(is2[:], sg2[:])
            ka = pp.tile([P, 1], F32, tag="ka")
            nc.vector.tensor_scalar(out=ka[:], in0=is2[:], scalar1=-K / 2.0,
                                    scalar2=None, op0=Alu.mult)
            ka2 = pp.tile([P, 1], F32, tag="ka2")
            nc.vector.tensor_scalar(out=ka2[:], in0=is2[:], scalar1=-K,
                                    scalar2=None, op0=Alu.mult)

            cyf = pp.tile([P, 1], F32, tag="cyf")
            nc.vector.tensor_copy(out=cyf[:], in_=cl[:, 0:1])
            cxf = pp.tile([P, 1], F32, tag="cxf")
            nc.vector.tensor_copy(out=cxf[:], in_=cl[:, 1:2])

            # const bias tiles for the scalar engine
            shiftc = pp.tile([P, 1], F32, tag="shiftc")
            nc.vector.memset(shiftc[:], SHIFT)
            lnfc = pp.tile([P, 1], F32, tag="lnfc")
            nc.vector.memset(lnfc[:], LN_FLOOR)

            # ---- separable gaussian factors U,V (and squared moment) ----
            io_i = pp.tile([P, W], I32, tag="io_i")
            if "noiota" in ablate:
                nc.vector.memset(io_i[:], 7)
            else:
                nc.gpsimd.iota(io_i[:], pattern=[[1, W]], base=0,
                               channel_multiplier=0)
            io_f = pp.tile([P, W], F32, tag="io_f")
            nc.vector.tensor_copy(out=io_f[:], in_=io_i[:])

            dy = pp.tile([P, W], F32, tag="dy")
            nc.vector.tensor_scalar(out=dy[:], in0=io_f[:], scalar1=cyf[:, 0:1],
                                    scalar2=None, op0=Alu.subtract)
            dy2 = pp.tile([P, W], F32, tag="dy2")
            nc.vector.tensor_tensor(out=dy2[:], in0=dy[:], in1=dy[:], op=Alu.mult)
            dx = pp.tile([P, W], F32, tag="dx")
            nc.vector.tensor_scalar(out=dx[:], in0=io_f[:], scalar1=cxf[:, 0:1],
                                    scalar2=None, op0=Alu.subtract)
            dx2 = pp.tile([P, W], F32, tag="dx2")
            nc.vector.tensor_tensor(out=dx2[:], in0=dx[:], in1=dx[:], op=Alu.mult)

            U = pp.tile([P, W], BF16, tag="U")
            nc.scalar.activation(out=U[:], in_=dy2[:], func=Act.Exp,
                                 bias=shiftc[:, 0:1], scale=ka[:, 0:1])
            U2 = pp.tile([P, W], BF16, tag="U2")
            nc.scalar.activation(out=U2[:], in_=dy2[:], func=Act.Exp,
                                 bias=shiftc[:, 0:1], scale=ka2[:, 0:1])
            V = pp.tile([P, W], BF16, tag="V")
            nc.scalar.activation(out=V[:], in_=dx2[:], func=Act.Exp,
                                 bias=shiftc[:, 0:1], scale=ka[:, 0:1])
            V2 = pp.tile([P, W], BF16, tag="V2")
            nc.scalar.activation(out=V2[:], in_=dx2[:], func=Act.Exp,
                                 bias=shiftc[:, 0:1], scale=ka2[:, 0:1])

            acc8 = pp.tile([P, 8], F32, tag="acc8")

            # ---- per-h-tile: matmul splat + epilogue + losses ----
            for t in range(NT):
                fs = slice(t * W, (t + 1) * W)
                rs = slice(t * P, (t + 1) * P)
                hslice = slice(t * P, (t + 1) * P)

                p16 = ps16.tile([P, W], F32, tag="p16")
                nc.tensor.matmul(out=p16[:], lhsT=U[:, hslice], rhs=V[:],
                                 start=True, stop=True)
                p32 = ps32.tile([P, W], F32, tag="p32")
                nc.tensor.matmul(out=p32[:], lhsT=U2[:, hslice], rhs=V2[:],
                                 start=True, stop=True)

                t1 = lp.tile([P, W], F32, tag="t1")
                nc.scalar.activation(out=t1[:], in_=p16[:], func=Act.Ln,
                                     bias=lnfc[:, 0:1])
                t2 = lp.tile([P, W], F32, tag="t2")
                nc.scalar.activation(out=t2[:], in_=p32[:], func=Act.Ln,
                                     bias=lnfc[:, 0:1])
                msel = lp.tile([P, W], F32, tag="msel")
                nc.vector.tensor_scalar(out=msel[:], in0=p32[:], scalar1=AIT_TH,
                                        scalar2=None, op0=Alu.is_gt)
                q1 = lp.tile([P, W], F32, tag="q1")
                nc.vector.scalar_tensor_tensor(
                    out=q1[:], in0=t1[:], scalar=-2.0, in1=t2[:],
                    op0=Alu.mult, op1=Alu.add)
                q2 = lp.tile([P, W], F32, tag="q2")
                nc.vector.scalar_tensor_tensor(
                    out=q2[:], in0=q1[:], scalar=UNSHIFT, in1=msel[:],
                    op0=Alu.add, op1=Alu.mult)
                z = lp.tile([P, W], F32, tag="z")
                nc.vector.scalar_tensor_tensor(
                    out=z[:], in0=t1[:], scalar=-UNSHIFT, in1=q2[:],
                    op0=Alu.add, op1=Alu.add)
                gts = lp.tile([P, W], F32, tag="gts")
                nc.scalar.activation(out=gts[:], in_=z[:], func=Act.Exp,
                                     scale=1.0 / K)
                nc.sync.dma_start(out=gt_e[rs, :], in_=gts[:])

                # scale loss partial: sum(sm^2) over this tile (ACT accum)
                scr = lp.tile([P, W], F32, tag="scr")
                nc.scalar.activation(out=scr[:], in_=smt[:, fs], func=Act.Square,
                                     accum_out=acc8[:, t:t + 1])

                # hm loss partial: sum((hm-gt)^2 * mask) over this tile
                d = lp.tile([P, W], F32, tag="d")
                nc.vector.tensor_tensor(out=d[:], in0=hmt[:, fs], in1=gts[:],
                                        op=Alu.subtract)
                dm = lp.tile([P, W], F32, tag="dm")
                nc.vector.tensor_tensor(out=dm[:], in0=d[:], in1=mkt[:, fs],
                                        op=Alu.mult)
                scr2 = lp.tile([P, W], F32, tag="scr2")
                nc.vector.tensor_tensor(out=scr2[:], in0=d[:], in1=dm[:],
                                        op=Alu.mult)
                nc.vector.reduce_sum(out=acc8[:, 4 + t:5 + t], in_=scr2[:],
                                     axis=mybir.AxisListType.X)

            # ---- cross-partition reduce of the 8 partials via matmul ----
            ones = pp.tile([P, 1], F32, tag="ones")
            nc.vector.memset(ones[:], 1.0)
            psr = psf.tile([8, 1], F32, tag="psr")
            nc.tensor.matmul(out=psr[:], lhsT=acc8[:], rhs=ones[:],
                             start=True, stop=True)
            part = pp.tile([8, 1], F32, tag="part")
            nc.scalar.activation(out=part[:], in_=psr[:], func=Act.Copy,
                                 scale=INV_HW)
            nc.sync.dma_start(out=pr_e[:], in_=part[:])

    nc.finalize()
    return nc


_NC = None


def _get_nc():
    global _NC
    if _NC is None:
        _NC = build_nc()
    return _NC


def make_in_maps(pred_hm, pred_sm, ground_resolution, mask, centers):
    in_maps = []
    for b in range(B):
        in_maps.append({
            "hm": np.ascontiguousarray(pred_hm[b, 0], dtype=np.float32),
            "sm": np.ascontiguousarray(pred_sm[b, 0], dtype=np.float32),
            "mask": np.ascontiguousarray(mask[b, 0], dtype=np.float32),
            "centers": np.ascontiguousarray(centers[b], dtype=np.int32),
            "grb": np.full((P, 1), ground_resolution[b], dtype=np.float32),
        })
    return in_maps


def run(pred_hm, pred_sm, ground_resolution, mask, centers, trace=False, **kw):
    nc = _get_nc()
    in_maps = make_in_maps(pred_hm, pred_sm, ground_resolution, mask, centers)
    res = run_bass_kernel_spmd(nc, in_maps, core_ids=list(range(B)),
                               trace=trace, **kw)
    gts = np.zeros((B, 1, H, W), np.float32)
    sls = np.zeros(B, np.float32)
    hls = np.zeros(B, np.float32)
    for b in range(B):
        out = res.results[b]
        gts[b, 0] = out["gt"]
        pr = out["partials"].reshape(8)
        sls[b] = np.float32(pr[0:4].sum(dtype=np.float32))
        hls[b] = np.float32(pr[4:8].sum(dtype=np.float32))
    sl = np.float32(sls.mean(dtype=np.float32))
    hl = np.float32(hls.mean(dtype=np.float32))
    return (sl, hl, gts), res


def kernel(pred_hm, pred_sm, ground_resolution, mask, centers):
    (sl, hl, gts), _ = run(pred_hm, pred_sm, ground_resolution, mask, centers)
    return sl, hl, gts
